# revision 6
# baseline (speedup 1.0000x reference)
"""TRN2 Bass kernel for nn_MultiHeadSeqAttention (B=8, M=1024, H=1024, 16 heads).

Reference computes out = ((h Wq^T) (h Wk^T)^T) (h Wv^T) per head, then Wo^T.
No softmax, so the product reassociates per head h:
    out_h = q_h @ (k_h^T v_h)      with  k_h^T v_h = Wk_h (h^T h) Wv_h^T
Route through the Gram matrix G = h^T h, which is SYMMETRIC:
    G  = hN^T hN          upper-triangle windows only: 72 MM-equivalents
                          (28 lower [128,128] blocks via 7 batched DMA-XBAR
                          transposes on the otherwise idle scalar queue)
    AT = G @ wk           [j, o], 128 MMs                  (wk = Wk^T)
    S'_h = wv_h^T AT_h    per-head [64,64], head-pair packed, 64 small MMs
    qT = wq^T ht          128 MMs
    R_h = S_h @ wo_h      32 quadrant-paired MMs   (folds Wo into S)
    out = qT^T @ R        128 MMs
vs the direct route (k, v, q, Wo GEMMs = 512 MMs + smalls) this saves ~56
big-MM equivalents (~12us PE) per core.

Schedule notes (from trace analysis):
 - every matmul lives in an 8-deep same-PSUM-bank accumulation group so
   LDWEIGHTS hides behind the previous matmul (216ns/MM); per-MM bank
   cycling measured 474ns/MM (LDW exposed + HAM oscillation).
 - all 6 input tensors stream on the sync queue as 7 descriptors in
   first-use order (hn halves, wk, wv, wq, ht, wo); DMA issue costs
   ~0.7us engine time each and the completion-semaphore pool is ~10 deep,
   so descriptor count is kept minimal.
 - G pass A is paced by the two hn half-arrivals (4-MM bursts per bank);
   a short const-operand warmup opens the HAM clock window beforehand.
 - output is bf16 (host casts back to f32), halving the store traffic.

Sharding: data-parallel over B across 8 cores; no collectives.
Precision: bf16 operands, fp32 PSUM accumulation; absmax rel err ~5.0e-3.
"""

import numpy as np
import ml_dtypes

import concourse.bass as bass
import concourse.mybir as mybir
import concourse.tile as tile
from concourse import bacc
from concourse.bass_utils import run_bass_kernel_spmd

F32 = mybir.dt.float32
BF16 = mybir.dt.bfloat16
COPY = mybir.ActivationFunctionType.Copy

P = 128          # partitions
H = 1024         # model dim
M = 1024         # sequence length
NT = H // P      # 8 tiles of 128
D = 64           # head dim
NC = 8           # cores
FD = 512         # matmul moving free dim (one PSUM bank of fp32)
WARMUP_MM = 8    # const-operand matmuls to open the HAM busy window

_CACHE = {}


def _build():
    nc = bacc.Bacc("TRN2", target_bir_lowering=False, debug=False,
                   num_devices=NC, enable_asserts=False)

    hn_d = nc.dram_tensor("hn", [M, H], BF16, kind="ExternalInput")
    wkb_d = nc.dram_tensor("wkb", [P, NT * H], BF16, kind="ExternalInput")
    wvb_d = nc.dram_tensor("wvb", [P, NT * H], BF16, kind="ExternalInput")
    wqb_d = nc.dram_tensor("wqb", [P, NT * H], BF16, kind="ExternalInput")
    htb_d = nc.dram_tensor("htb", [P, NT * H], BF16, kind="ExternalInput")
    wob_d = nc.dram_tensor("wob", [P, NT * H], BF16, kind="ExternalInput")
    out_d = nc.dram_tensor("out", [M, H], BF16, kind="ExternalOutput")

    with tile.TileContext(nc) as tc:
        with tc.tile_pool(name="sb", bufs=1) as sb, \
             tc.tile_pool(name="ps", bufs=1, space="PSUM") as ps:

            # ---- warmup: dep-free matmuls; ends about when hn half 0 lands
            wu_lhs = nc.const_aps.tensor(1.0, [P, P], BF16)
            wu_rhs = nc.const_aps.tensor(1.0, [P, FD], BF16)
            wu_ps = ps.tile([P, FD], F32, tag="big", bufs=8, name="wu_ps")
            for _ in range(WARMUP_MM):
                nc.tensor.matmul(wu_ps[:], wu_lhs, wu_rhs,
                                 start=True, stop=True)

            # ---- loads. hn halves are split across BOTH hwdge queues (a
            # single queue sustains only ~235 GB/s); weights follow in
            # first-use order: sync gets wk/wq/ht/wo, scalar gets wv (its
            # queue then runs the G transposes). ----
            hnA = sb.tile([P, 4 * H], BF16, tag="hnA", name="hnA")
            hnB = sb.tile([P, 4 * H], BF16, tag="hnB", name="hnB")
            for half, t in ((0, hnA), (1, hnB)):
                for qi, eng in ((0, nc.sync), (1, nc.scalar)):
                    r0 = 512 * half + 256 * qi
                    src = hn_d.ap()[r0:r0 + 256, :] \
                        .rearrange("(t p) c -> p t c", p=P)
                    dst = t[:, 2 * H * qi:2 * H * qi + 2 * H] \
                        .rearrange("p (t c) -> p t c", c=H)
                    eng.dma_start(dst, src)

            def hn(s):
                t = hnA if s < 4 else hnB
                return t[:, H * (s % 4):H * (s % 4) + H]

            def big_load(dram, tag, eng):
                t = sb.tile([P, NT * H], BF16, tag=tag, name=tag)
                eng.dma_start(t[:], dram.ap()[:, :])
                return t

            wkB = big_load(wkb_d, "wkB", nc.sync)
            wvB = big_load(wvb_d, "wvB", nc.scalar)
            wqB = big_load(wqb_d, "wqB", nc.sync)
            htB = big_load(htb_d, "htB", nc.sync)
            woB = big_load(wob_d, "woB", nc.sync)

            # ---- phase G: upper-triangle windows of G = hN^T hN.
            # Row-chunk ib keeps columns [128*ib, 1024) as (up to) two PSUM
            # windows. Pass A = ib 0-3 (8 banks), 4-MM bursts per bank paced
            # by the hn halves; pass B = ib 4-7 (4 banks), dense. ----
            gt = sb.tile([P, NT * H], BF16, tag="gt", name="gt")
            WIN_A = []
            for ib in range(4):
                WIN_A.append((ib, P * ib, FD))
                WIN_A.append((ib, P * ib + FD, FD - P * ib))
            gpsA = {}
            for (ib, c0, w) in WIN_A:
                gpsA[(ib, c0)] = ps.tile([P, w], F32, tag="big", bufs=8,
                                         name=f"gA{ib}_{c0}")
            for half in range(2):
                for (ib, c0, w) in WIN_A:
                    for u in range(4):
                        s = 4 * half + u
                        nc.tensor.matmul(
                            gpsA[(ib, c0)][:],
                            hn(s)[:, P * ib:P * ib + P],
                            hn(s)[:, c0:c0 + w],
                            start=(s == 0), stop=(s == NT - 1),
                            skip_group_check=True,
                        )
            # casts ordered ib 3..0 so transpose sources complete earliest
            for (ib, c0, w) in sorted(WIN_A, key=lambda x: -x[0]):
                nc.vector.tensor_copy(gt[:, H * ib + c0:H * ib + c0 + w],
                                      gpsA[(ib, c0)][:])
            for ib in range(4, 8):
                c0, w = P * ib, H - P * ib
                pt = ps.tile([P, w], F32, tag="big", bufs=8, name=f"gB{ib}")
                for s in range(NT):
                    nc.tensor.matmul(
                        pt[:],
                        hn(s)[:, P * ib:P * ib + P],
                        hn(s)[:, c0:c0 + w],
                        start=(s == 0), stop=(s == NT - 1),
                    )
                nc.vector.tensor_copy(gt[:, H * ib + c0:H * ib + c0 + w],
                                      pt[:])

            # lower-triangle blocks (b, a), b > a: batched XBAR transposes on
            # the scalar queue, one instruction per source row-chunk a,
            # ordered by when the AT phase consumes them (jb visit order).
            gt3 = gt[:].rearrange("p (b c) -> p b c", c=H)
            for a in (3, 2, 1, 0, 6, 5, 4):
                nc.scalar.dma_start(
                    gt3[:, a + 1:NT, P * a:P * a + P],
                    gt[:, H * a + P * (a + 1):H * a + H],
                    transpose=True)

            # ---- phase AT + S': AT = G @ wk; S' pairs accumulate after each
            # at-tile half is cast. jb=7 needs no transposed blocks; jb 3..0
            # need pass-A-sourced transposes, jb 6..4 pass-B-sourced. ----
            s_psA = ps.tile([P, FD], F32, tag="big", bufs=8, name="s_psA")
            s_psB = ps.tile([P, FD], F32, tag="big", bufs=8, name="s_psB")
            nc.vector.memset(s_psA[:], 0.0)
            nc.vector.memset(s_psB[:], 0.0)

            # jb=3 first: its transposed blocks come from pass-A casts alone
            # (transpose a=3 fires before pass B even finishes); jb 7..4 last.
            AT_ORDER = [3, 2, 1, 0, 7, 6, 5, 4]
            for idx, jb in enumerate(AT_ORDER):
                a_t = sb.tile([P, H], BF16, tag="at", bufs=3, name=f"at{jb}")
                for oc in range(2):
                    p_t = ps.tile([P, FD], F32, tag="big", bufs=8,
                                  name=f"pa{jb}{oc}")
                    for ib in range(NT):
                        nc.tensor.matmul(
                            p_t[:],
                            gt[:, H * ib + P * jb:H * ib + P * jb + P],
                            wkB[:, H * ib + FD * oc:H * ib + FD * oc + FD],
                            start=(ib == 0), stop=(ib == NT - 1),
                        )
                    nc.vector.tensor_copy(a_t[:, FD * oc:FD * oc + FD], p_t[:])
                    for g in range(4 * oc, 4 * oc + 4):
                        bank = s_psA if g < 4 else s_psB
                        cc = P * (g % 4)
                        nc.tensor.matmul(
                            bank[:, cc:cc + P],
                            wvB[:, H * jb + P * g:H * jb + P * g + P],
                            a_t[:, P * g:P * g + P],
                            start=False, stop=(idx == NT - 1),
                            skip_group_check=True,
                        )
            s_sbA = sb.tile([P, FD], BF16, tag="ssb", bufs=2, name="s_sbA")
            s_sbB = sb.tile([P, FD], BF16, tag="ssb", bufs=2, name="s_sbB")
            nc.scalar.activation(s_sbA[:], s_psA[:], COPY)
            nc.scalar.activation(s_sbB[:], s_psB[:], COPY)

            # ---- phase qT + R: qT = wq^T ht; R_g folds Wo into S per head
            # pair (quadrant-packed 64-part matmuls), spread between q tiles ----
            qt = [None] * NT
            rstack = [None] * NT

            def emit_q(to):
                q_t = sb.tile([P, M], BF16, tag=f"qt{to}", name=f"qt{to}")
                for cm in range(2):
                    p_t = ps.tile([P, FD], F32, tag="big", bufs=8,
                                  name=f"pq{to}{cm}")
                    for ci in range(NT):
                        nc.tensor.matmul(
                            p_t[:],
                            wqB[:, H * ci + P * to:H * ci + P * to + P],
                            htB[:, H * ci + FD * cm:H * ci + FD * cm + FD],
                            start=(ci == 0), stop=(ci == NT - 1),
                        )
                    nc.vector.tensor_copy(q_t[:, FD * cm:FD * cm + FD], p_t[:])
                qt[to] = q_t

            def emit_r(g):
                r_t = sb.tile([P, H], BF16, tag=f"rs{g}", name=f"rs{g}")
                sbank = s_sbA if g < 4 else s_sbB
                cc = P * (g % 4)
                for jc in range(2):
                    p_t = ps.tile([P, FD], F32, tag="big", bufs=8,
                                  name=f"pr{g}{jc}")
                    for hh in range(2):
                        pb = D * hh
                        nc.tensor.matmul(
                            p_t[pb:pb + D, :],
                            sbank[pb:pb + D, cc + pb:cc + pb + D],
                            woB[pb:pb + D, H * g + FD * jc:H * g + FD * jc + FD],
                            start=True, stop=True,
                        )
                    nc.scalar.activation(r_t[:, FD * jc:FD * jc + FD],
                                         p_t[:], COPY)
                rstack[g] = r_t

            for to in range(NT):
                emit_q(to)
                emit_r(to)

            # ---- phase out: out = qT^T @ R ----
            for tm in range(NT):
                o_sb = sb.tile([P, H], BF16, tag="ot", bufs=3, name=f"o{tm}")
                for cj in range(2):
                    p_t = ps.tile([P, FD], F32, tag="big", bufs=8,
                                  name=f"pf{tm}{cj}")
                    for to in range(NT):
                        nc.tensor.matmul(
                            p_t[:],
                            qt[to][:, P * tm:P * tm + P],
                            rstack[to][:, FD * cj:FD * cj + FD],
                            start=(to == 0), stop=(to == NT - 1),
                        )
                    if tm == NT - 1:
                        # last tile: quarter-chunk casts+stores so the end of
                        # kernel gates on a 64KB transfer
                        for qq in range(2):
                            off = FD * cj + 256 * qq
                            nc.vector.tensor_copy(
                                o_sb[:, off:off + 256],
                                p_t[:, 256 * qq:256 * qq + 256])
                            (nc.sync if qq else nc.scalar).dma_start(
                                out_d.ap()[P * tm:P * tm + P, off:off + 256],
                                o_sb[:, off:off + 256])
                    else:
                        nc.vector.tensor_copy(o_sb[:, FD * cj:FD * cj + FD],
                                              p_t[:])
                if tm < NT - 1:
                    nc.sync.dma_start(out_d.ap()[P * tm:P * tm + P, :],
                                      o_sb[:])

    nc.compile()
    return nc


def _get_nc():
    if "nc" not in _CACHE:
        _CACHE["nc"] = _build()
    return _CACHE["nc"]


def _arrange(wt_f32):
    """[NT*P, H] row-tile layout -> [P, NT*H] one-descriptor layout."""
    bf16 = ml_dtypes.bfloat16
    a = np.ascontiguousarray(wt_f32).astype(bf16)
    return np.ascontiguousarray(
        a.reshape(NT, P, H).transpose(1, 0, 2).reshape(P, NT * H))


def _run(h, Wq, Wk, Wv, Wo, trace=False):
    nc = _get_nc()
    bf16 = ml_dtypes.bfloat16
    wkb = _arrange(np.asarray(Wk).T)
    wvb = _arrange(np.asarray(Wv).T)
    wqb = _arrange(np.asarray(Wq).T)
    wob = _arrange(np.asarray(Wo).T)
    in_maps = []
    for b in range(NC):
        hb = np.ascontiguousarray(np.asarray(h[b])).astype(bf16)
        htb = _arrange(np.asarray(h[b]).T)
        in_maps.append({
            "hn": hb, "htb": htb,
            "wkb": wkb, "wvb": wvb, "wqb": wqb, "wob": wob,
        })
    res = run_bass_kernel_spmd(nc, in_maps, core_ids=list(range(NC)),
                               trace=trace)
    out = np.stack(
        [res.results[b]["out"].astype(np.float32) for b in range(NC)], axis=0)
    return out, res


def kernel(h, key_pe, Wq, Wk, Wv, Wo):
    # key_pe only feeds the reference's dead softmax branch; unused.
    out, _ = _run(h, Wq, Wk, Wv, Wo)
    return out


# revision 7
# speedup vs baseline: 1.0431x; 1.0431x over previous
"""TRN2 Bass kernel for nn_MultiHeadSeqAttention (B=8, M=1024, H=1024, 16 heads).

Reference computes out = ((h Wq^T) (h Wk^T)^T) (h Wv^T) per head, then Wo^T.
No softmax, so the product reassociates per head h:
    out_h = q_h @ (k_h^T v_h)      with  k_h^T v_h = Wk_h (h^T h) Wv_h^T
Route through the Gram matrix G = h^T h, which is SYMMETRIC:
    G  = hN^T hN          upper-triangle windows only: 72 MM-equivalents
                          (28 lower [128,128] blocks via 7 batched DMA-XBAR
                          transposes on the otherwise idle scalar queue)
    AT = G @ wk           [j, o], 128 MMs                  (wk = Wk^T)
    S'_h = wv_h^T AT_h    per-head [64,64], head-pair packed, 64 small MMs
    qT = wq^T ht          128 MMs
    R_h = S_h @ wo_h      32 quadrant-paired MMs   (folds Wo into S)
    out = qT^T @ R        128 MMs
vs the direct route (k, v, q, Wo GEMMs = 512 MMs + smalls) this saves ~56
big-MM equivalents (~12us PE) per core.

Schedule notes (from trace analysis):
 - every matmul lives in an 8-deep same-PSUM-bank accumulation group so
   LDWEIGHTS hides behind the previous matmul (216ns/MM); per-MM bank
   cycling measured 474ns/MM (LDW exposed + HAM oscillation).
 - all 6 input tensors stream on the sync queue as 7 descriptors in
   first-use order (hn halves, wk, wv, wq, ht, wo); DMA issue costs
   ~0.7us engine time each and the completion-semaphore pool is ~10 deep,
   so descriptor count is kept minimal.
 - G pass A is paced by the two hn half-arrivals (4-MM bursts per bank);
   a short const-operand warmup opens the HAM clock window beforehand.
 - output is bf16 (host casts back to f32), halving the store traffic.

Sharding: data-parallel over B across 8 cores; no collectives.
Precision: bf16 operands, fp32 PSUM accumulation; absmax rel err ~5.0e-3.
"""

import numpy as np
import ml_dtypes

import concourse.bass as bass
import concourse.mybir as mybir
import concourse.tile as tile
from concourse import bacc
from concourse.bass_utils import run_bass_kernel_spmd

F32 = mybir.dt.float32
BF16 = mybir.dt.bfloat16
COPY = mybir.ActivationFunctionType.Copy

P = 128          # partitions
H = 1024         # model dim
M = 1024         # sequence length
NT = H // P      # 8 tiles of 128
D = 64           # head dim
NC = 8           # cores
FD = 512         # matmul moving free dim (one PSUM bank of fp32)
WARMUP_MM = 8    # const-operand matmuls to open the HAM busy window

_CACHE = {}


def _build():
    nc = bacc.Bacc("TRN2", target_bir_lowering=False, debug=False,
                   num_devices=NC, enable_asserts=False)

    hn_d = nc.dram_tensor("hn", [M, H], BF16, kind="ExternalInput")
    wkb_d = nc.dram_tensor("wkb", [P, NT * H], BF16, kind="ExternalInput")
    wvb_d = nc.dram_tensor("wvb", [P, NT * H], BF16, kind="ExternalInput")
    wqb_d = nc.dram_tensor("wqb", [P, NT * H], BF16, kind="ExternalInput")
    htb_d = nc.dram_tensor("htb", [P, NT * H], BF16, kind="ExternalInput")
    wob_d = nc.dram_tensor("wob", [P, NT * H], BF16, kind="ExternalInput")
    out_d = nc.dram_tensor("out", [M, H], BF16, kind="ExternalOutput")

    with tile.TileContext(nc) as tc:
        with tc.tile_pool(name="sb", bufs=1) as sb, \
             tc.tile_pool(name="ps", bufs=1, space="PSUM") as ps:

            # ---- warmup: dep-free matmuls; ends about when hn half 0 lands
            wu_lhs = nc.const_aps.tensor(1.0, [P, P], BF16)
            wu_rhs = nc.const_aps.tensor(1.0, [P, FD], BF16)
            wu_ps = ps.tile([P, FD], F32, tag="big", bufs=8, name="wu_ps")
            for _ in range(WARMUP_MM):
                nc.tensor.matmul(wu_ps[:], wu_lhs, wu_rhs,
                                 start=True, stop=True)

            # ---- loads. hn halves are split across BOTH hwdge queues (a
            # single queue sustains only ~235 GB/s); weights follow in
            # first-use order: sync gets wk/wq/ht/wo, scalar gets wv (its
            # queue then runs the G transposes). ----
            hnA = sb.tile([P, 4 * H], BF16, tag="hnA", name="hnA")
            hnB = sb.tile([P, 4 * H], BF16, tag="hnB", name="hnB")
            for half, t in ((0, hnA), (1, hnB)):
                for qi, eng in ((0, nc.sync), (1, nc.scalar)):
                    r0 = 512 * half + 256 * qi
                    src = hn_d.ap()[r0:r0 + 256, :] \
                        .rearrange("(t p) c -> p t c", p=P)
                    dst = t[:, 2 * H * qi:2 * H * qi + 2 * H] \
                        .rearrange("p (t c) -> p t c", c=H)
                    eng.dma_start(dst, src)

            def hn(s):
                t = hnA if s < 4 else hnB
                return t[:, H * (s % 4):H * (s % 4) + H]

            def big_load(dram, tag, eng):
                t = sb.tile([P, NT * H], BF16, tag=tag, name=tag)
                eng.dma_start(t[:], dram.ap()[:, :])
                return t

            wkB = big_load(wkb_d, "wkB", nc.sync)
            wvB = big_load(wvb_d, "wvB", nc.scalar)
            wqB = big_load(wqb_d, "wqB", nc.sync)
            htB = big_load(htb_d, "htB", nc.sync)
            woB = big_load(wob_d, "woB", nc.sync)

            # ---- phase G: upper-triangle windows of G = hN^T hN.
            # Row-chunk ib keeps columns [128*ib, 1024) as (up to) two PSUM
            # windows. Pass A = ib 0-3 (8 banks), 4-MM bursts per bank paced
            # by the hn halves; pass B = ib 4-7 (4 banks), dense. ----
            gt = sb.tile([P, NT * H], BF16, tag="gt", name="gt")
            WIN_A = []
            for ib in range(4):
                WIN_A.append((ib, P * ib, FD))
                WIN_A.append((ib, P * ib + FD, FD - P * ib))
            gpsA = {}
            for (ib, c0, w) in WIN_A:
                gpsA[(ib, c0)] = ps.tile([P, w], F32, tag="big", bufs=8,
                                         name=f"gA{ib}_{c0}")
            # half1 runs ib 3 first so cast(3) -> transpose a=3 -> AT jb=3
            # become ready in exactly the order the AT phase consumes them
            # (the tile scheduler follows dependency readiness, not emission)
            for half, accs in ((0, WIN_A), (1, list(reversed(WIN_A)))):
                for (ib, c0, w) in accs:
                    for u in range(4):
                        s = 4 * half + u
                        nc.tensor.matmul(
                            gpsA[(ib, c0)][:],
                            hn(s)[:, P * ib:P * ib + P],
                            hn(s)[:, c0:c0 + w],
                            start=(s == 0), stop=(s == NT - 1),
                            skip_group_check=True,
                        )
            for (ib, c0, w) in sorted(WIN_A, key=lambda x: -x[0]):
                nc.vector.tensor_copy(gt[:, H * ib + c0:H * ib + c0 + w],
                                      gpsA[(ib, c0)][:])
            for ib in range(4, 8):
                c0, w = P * ib, H - P * ib
                pt = ps.tile([P, w], F32, tag="big", bufs=8, name=f"gB{ib}")
                for s in range(NT):
                    nc.tensor.matmul(
                        pt[:],
                        hn(s)[:, P * ib:P * ib + P],
                        hn(s)[:, c0:c0 + w],
                        start=(s == 0), stop=(s == NT - 1),
                    )
                nc.vector.tensor_copy(gt[:, H * ib + c0:H * ib + c0 + w],
                                      pt[:])

            # lower-triangle blocks (b, a), b > a: batched XBAR transposes on
            # the scalar queue, one instruction per source row-chunk a,
            # ordered by when the AT phase consumes them (jb visit order).
            gt3 = gt[:].rearrange("p (b c) -> p b c", c=H)
            for a in (3, 2, 1, 0, 6, 5, 4):
                nc.scalar.dma_start(
                    gt3[:, a + 1:NT, P * a:P * a + P],
                    gt[:, H * a + P * (a + 1):H * a + H],
                    transpose=True)

            # ---- phase AT + S': AT = G @ wk; S' pairs accumulate after each
            # at-tile half is cast. jb=7 needs no transposed blocks; jb 3..0
            # need pass-A-sourced transposes, jb 6..4 pass-B-sourced. ----
            s_psA = ps.tile([P, FD], F32, tag="big", bufs=8, name="s_psA")
            s_psB = ps.tile([P, FD], F32, tag="big", bufs=8, name="s_psB")
            nc.vector.memset(s_psA[:], 0.0)
            nc.vector.memset(s_psB[:], 0.0)

            # jb=3 first: its transposed blocks come from pass-A casts alone
            # (transpose a=3 fires before pass B even finishes); jb 7..4 last.
            AT_ORDER = [3, 2, 1, 0, 7, 6, 5, 4]
            for idx, jb in enumerate(AT_ORDER):
                a_t = sb.tile([P, H], BF16, tag="at", bufs=3, name=f"at{jb}")
                for oc in range(2):
                    p_t = ps.tile([P, FD], F32, tag="big", bufs=8,
                                  name=f"pa{jb}{oc}")
                    for ib in range(NT):
                        nc.tensor.matmul(
                            p_t[:],
                            gt[:, H * ib + P * jb:H * ib + P * jb + P],
                            wkB[:, H * ib + FD * oc:H * ib + FD * oc + FD],
                            start=(ib == 0), stop=(ib == NT - 1),
                        )
                    nc.vector.tensor_copy(a_t[:, FD * oc:FD * oc + FD], p_t[:])
                    for g in range(4 * oc, 4 * oc + 4):
                        bank = s_psA if g < 4 else s_psB
                        cc = P * (g % 4)
                        nc.tensor.matmul(
                            bank[:, cc:cc + P],
                            wvB[:, H * jb + P * g:H * jb + P * g + P],
                            a_t[:, P * g:P * g + P],
                            start=False, stop=(idx == NT - 1),
                            skip_group_check=True,
                        )
            s_sbA = sb.tile([P, FD], BF16, tag="ssb", bufs=2, name="s_sbA")
            s_sbB = sb.tile([P, FD], BF16, tag="ssb", bufs=2, name="s_sbB")
            nc.scalar.activation(s_sbA[:], s_psA[:], COPY)
            nc.scalar.activation(s_sbB[:], s_psB[:], COPY)

            # ---- phase qT + R: qT = wq^T ht; R_g folds Wo into S per head
            # pair (quadrant-packed 64-part matmuls), spread between q tiles ----
            qt = [None] * NT
            rstack = [None] * NT

            def emit_q(to):
                q_t = sb.tile([P, M], BF16, tag=f"qt{to}", name=f"qt{to}")
                for cm in range(2):
                    p_t = ps.tile([P, FD], F32, tag="big", bufs=8,
                                  name=f"pq{to}{cm}")
                    for ci in range(NT):
                        nc.tensor.matmul(
                            p_t[:],
                            wqB[:, H * ci + P * to:H * ci + P * to + P],
                            htB[:, H * ci + FD * cm:H * ci + FD * cm + FD],
                            start=(ci == 0), stop=(ci == NT - 1),
                        )
                    nc.vector.tensor_copy(q_t[:, FD * cm:FD * cm + FD], p_t[:])
                qt[to] = q_t

            def emit_r(g):
                r_t = sb.tile([P, H], BF16, tag=f"rs{g}", name=f"rs{g}")
                sbank = s_sbA if g < 4 else s_sbB
                cc = P * (g % 4)
                for jc in range(2):
                    p_t = ps.tile([P, FD], F32, tag="big", bufs=8,
                                  name=f"pr{g}{jc}")
                    for hh in range(2):
                        pb = D * hh
                        nc.tensor.matmul(
                            p_t[pb:pb + D, :],
                            sbank[pb:pb + D, cc + pb:cc + pb + D],
                            woB[pb:pb + D, H * g + FD * jc:H * g + FD * jc + FD],
                            start=True, stop=True,
                        )
                    nc.scalar.activation(r_t[:, FD * jc:FD * jc + FD],
                                         p_t[:], COPY)
                rstack[g] = r_t

            for to in range(NT):
                emit_q(to)
                emit_r(to)

            # ---- phase out: out = qT^T @ R ----
            for tm in range(NT):
                o_sb = sb.tile([P, H], BF16, tag="ot", bufs=3, name=f"o{tm}")
                for cj in range(2):
                    p_t = ps.tile([P, FD], F32, tag="big", bufs=8,
                                  name=f"pf{tm}{cj}")
                    for to in range(NT):
                        nc.tensor.matmul(
                            p_t[:],
                            qt[to][:, P * tm:P * tm + P],
                            rstack[to][:, FD * cj:FD * cj + FD],
                            start=(to == 0), stop=(to == NT - 1),
                        )
                    if tm == NT - 1:
                        # last tile: quarter-chunk casts+stores so the end of
                        # kernel gates on a 64KB transfer
                        for qq in range(2):
                            off = FD * cj + 256 * qq
                            nc.vector.tensor_copy(
                                o_sb[:, off:off + 256],
                                p_t[:, 256 * qq:256 * qq + 256])
                            (nc.sync if qq else nc.scalar).dma_start(
                                out_d.ap()[P * tm:P * tm + P, off:off + 256],
                                o_sb[:, off:off + 256])
                    else:
                        nc.vector.tensor_copy(o_sb[:, FD * cj:FD * cj + FD],
                                              p_t[:])
                if tm < NT - 1:
                    nc.sync.dma_start(out_d.ap()[P * tm:P * tm + P, :],
                                      o_sb[:])

    nc.compile()
    return nc


def _get_nc():
    if "nc" not in _CACHE:
        _CACHE["nc"] = _build()
    return _CACHE["nc"]


def _arrange(wt_f32):
    """[NT*P, H] row-tile layout -> [P, NT*H] one-descriptor layout."""
    bf16 = ml_dtypes.bfloat16
    a = np.ascontiguousarray(wt_f32).astype(bf16)
    return np.ascontiguousarray(
        a.reshape(NT, P, H).transpose(1, 0, 2).reshape(P, NT * H))


def _run(h, Wq, Wk, Wv, Wo, trace=False):
    nc = _get_nc()
    bf16 = ml_dtypes.bfloat16
    wkb = _arrange(np.asarray(Wk).T)
    wvb = _arrange(np.asarray(Wv).T)
    wqb = _arrange(np.asarray(Wq).T)
    wob = _arrange(np.asarray(Wo).T)
    in_maps = []
    for b in range(NC):
        hb = np.ascontiguousarray(np.asarray(h[b])).astype(bf16)
        htb = _arrange(np.asarray(h[b]).T)
        in_maps.append({
            "hn": hb, "htb": htb,
            "wkb": wkb, "wvb": wvb, "wqb": wqb, "wob": wob,
        })
    res = run_bass_kernel_spmd(nc, in_maps, core_ids=list(range(NC)),
                               trace=trace)
    out = np.stack(
        [res.results[b]["out"].astype(np.float32) for b in range(NC)], axis=0)
    return out, res


def kernel(h, key_pe, Wq, Wk, Wv, Wo):
    # key_pe only feeds the reference's dead softmax branch; unused.
    out, _ = _run(h, Wq, Wk, Wv, Wo)
    return out
